# revision 16
# baseline (speedup 1.0000x reference)
# Bass/Trainium2 kernel for nn_CPRPackedLinear (mixed 6-bit/5-bit packed
# quantized linear layer), tensor-parallel over out_features on 8 NeuronCores.
#
# Math: out = x_perm[:, :1024] @ deq6(W_high) + x_perm[:, 1024:] @ deq5(W_low) + bias
# where deq = (unpack_bits(packed) - half_range) * group_scale.
#
# Device strategy (per core, N-shard of 1376 columns):
#   The bit-unpack is decomposed into "planes": each packed byte b contributes
#   bitfields ((b >> s) or (b & m)) that enter the matmul as extra contraction
#   rows, with the recombination coefficients folded into a host-prepared
#   duplicated-x stationary operand. Each plane is ONE fused DVE op
#   (scalar_tensor_tensor): plane = (bytes op imm) * scale_tile, which also
#   applies the per-(group, column) dequant scale. The (-half*scale) offsets
#   and the bias enter via one small correction matmul. A single PSUM
#   accumulation chain per 512-column chunk sums everything.
import numpy as np
import ml_dtypes

import concourse.bass as bass
import concourse.mybir as mybir
from concourse import bacc
from concourse.tile import TileContext
from concourse.bass_utils import run_bass_kernel_spmd

BF16 = ml_dtypes.bfloat16

N_CORES = 8
M = 64
OUT_FEATURES = 11008
N_PER = OUT_FEATURES // N_CORES  # 1376
N_HIGH = 1024
N_LOW = 3072
GROUP = 128

# (byte_idx, op, imm, [(value_offset_j, coeff), ...])
# 'raw' planes are the bytes themselves (no bit op needed); masked fields are
# recovered linearly: b>>c = (b - (b & (2^c-1))) / 2^c, folded into coeffs.
# raw planes first (they batch into one op per K-tile and unblock the PE
# earliest), then the masked planes
HIGH_PLANES = [
    (0, 'raw', 0, [(1, 1 / 64)]),
    (1, 'raw', 0, [(2, 1 / 16)]),
    (2, 'raw', 0, [(3, 1 / 4)]),
    (0, 'and', 63, [(0, 1.0), (1, -1 / 64)]),
    (1, 'and', 15, [(1, 4.0), (2, -1 / 16)]),
    (2, 'and', 3, [(2, 16.0), (3, -1 / 4)]),
]
LOW_PLANES = [
    (0, 'raw', 0, [(1, 1 / 32)]),
    (1, 'raw', 0, [(2, 1 / 4)]),
    (2, 'raw', 0, [(4, 1 / 16)]),
    (3, 'raw', 0, [(6, 1 / 64)]),
    (4, 'raw', 0, [(7, 1 / 8)]),
    (0, 'and', 31, [(0, 1.0), (1, -1 / 32)]),
    (1, 'and', 3, [(1, 8.0), (2, -1 / 4)]),
    # bit 7 of b1 via an arith compare: one fused (b>=128)*scale op
    (1, 'ge', 128, [(3, 1.0), (2, -32.0)]),
    (2, 'and', 15, [(3, 2.0), (4, -1 / 16)]),
    (3, 'and', 1, [(4, 16.0), (5, -1 / 2)]),
    (3, 'and', 63, [(5, 1 / 2), (6, -1 / 64)]),
    (4, 'and', 7, [(6, 4.0), (7, -1 / 8)]),
]
# first K-tile: per-byte-row order (follows DMA arrival), no raw batching
HIGH_PLANES_KT0 = [
    (0, 'raw', 0, [(1, 1 / 64)]),
    (0, 'and', 63, [(0, 1.0), (1, -1 / 64)]),
    (1, 'raw', 0, [(2, 1 / 16)]),
    (1, 'and', 15, [(1, 4.0), (2, -1 / 16)]),
    (2, 'raw', 0, [(3, 1 / 4)]),
    (2, 'and', 3, [(2, 16.0), (3, -1 / 4)]),
]

KTILES = [('high', 0), ('high', 1), ('low', 0), ('low', 1), ('low', 2)]
N_BLOCKS = 2 * len(HIGH_PLANES) + 3 * len(LOW_PLANES)  # 48
NCHUNKS = [(1024, 352), (0, 512), (512, 512)]


def ktile_planes(kt, region):
    if kt == 0:
        return HIGH_PLANES_KT0
    return HIGH_PLANES if region == 'high' else LOW_PLANES

_ALU = {
    'and': mybir.AluOpType.bitwise_and,
    'shr': mybir.AluOpType.logical_shift_right,
}


def build_nc():
    nc = bacc.Bacc(None, name="cpr_packed_linear", enable_partition_id=False)
    f32 = mybir.dt.float32
    bf16 = mybir.dt.bfloat16
    i16 = mybir.dt.int16

    bh = nc.dram_tensor("bytes_high", [128, 2, 3, N_PER], i16, kind="ExternalInput")
    bl = nc.dram_tensor("bytes_low", [128, 3, 5, N_PER], i16, kind="ExternalInput")
    st = nc.dram_tensor("scale_tiles", [128, 5, N_PER], bf16, kind="ExternalInput")
    xd = nc.dram_tensor("xdup", [128, N_BLOCKS, M], bf16, kind="ExternalInput")
    cl = nc.dram_tensor("corr_lhsT", [33, M], f32, kind="ExternalInput")
    cr = nc.dram_tensor("corr_rhs", [33, N_PER], f32, kind="ExternalInput")
    out = nc.dram_tensor("out", [M, N_PER], f32, kind="ExternalOutput")

    with TileContext(nc) as tc, \
         tc.tile_pool(name="const", bufs=1) as const_pool, \
         tc.tile_pool(name="bytes", bufs=3) as bytes_pool, \
         tc.tile_pool(name="scales", bufs=2) as scale_pool, \
         tc.tile_pool(name="planes", bufs=2) as plane_pool, \
         tc.tile_pool(name="psum", bufs=1, space="PSUM") as psum_pool:

        # small/critical tensors first, first K-tile bytes split per byte-row
        # so the first plane can start after ~1/3 of the transfer
        byt0 = bytes_pool.tile([128, 3, N_PER], i16, tag="bytes", name="byt0")
        nc.sync.dma_start(out=byt0[:, 0], in_=bh[:, 0, 0])
        sct0 = scale_pool.tile([128, N_PER], bf16, tag="sc", name="sct0")
        nc.scalar.dma_start(out=sct0[:], in_=st[:, 0])
        xd_t = const_pool.tile([128, N_BLOCKS, M], bf16, tag="xd")
        nc.scalar.dma_start(out=xd_t[:, :6], in_=xd[:, :6])
        nc.sync.dma_start(out=byt0[:, 1], in_=bh[:, 0, 1])
        nc.sync.dma_start(out=byt0[:, 2], in_=bh[:, 0, 2])
        nc.scalar.dma_start(out=xd_t[:, 6:], in_=xd[:, 6:])
        cl_t = const_pool.tile([33, M], f32, tag="cl")
        nc.scalar.dma_start(out=cl_t[:], in_=cl[:])
        cr_t = const_pool.tile([33, N_PER], f32, tag="cr")
        nc.scalar.dma_start(out=cr_t[:], in_=cr[:])
        out_sb = const_pool.tile([M, N_PER], f32, tag="outsb")

        psums = [psum_pool.tile([M, w], mybir.dt.float32, tag=f"ps{i}",
                                name=f"ps{i}")
                 for i, (o, w) in enumerate(NCHUNKS)]

        bi = 0
        for kt, (region, t) in enumerate(KTILES):
            nb = 3 if region == 'high' else 5
            if kt == 0:
                byt, sct_full = byt0, sct0
            else:
                byt = bytes_pool.tile([128, nb, N_PER], i16, tag="bytes",
                                      name="byt")
                src = bh[:, t] if region == 'high' else bl[:, t]
                # per-byte-row DMAs: each masked extract can start as soon as
                # its row lands instead of waiting for the whole tile
                for i in range(nb):
                    nc.sync.dma_start(out=byt[:, i], in_=src[:, i])
                sct_full = scale_pool.tile([128, N_PER], bf16, tag="sc",
                                           name="sct")
                nc.scalar.dma_start(out=sct_full[:], in_=st[:, kt])
            sct = sct_full[:]

            planes = ktile_planes(kt, region)

            if kt != 0:
                # all raw planes in ONE batched DVE op: bytes * scale with
                # the scale broadcast (step-0) over the byte-row dim
                rawt = plane_pool.tile([128, nb, N_PER], bf16, tag="rawt",
                                       name="rawt", bufs=3)
                sct1 = sct_full[:].rearrange("p (o n) -> p o n", o=1)
                nc.vector.tensor_tensor(rawt[:], byt[:],
                                        sct1.broadcast_to([128, nb, N_PER]),
                                        mybir.AluOpType.mult)

            for (i, op, imm, _coeffs) in planes:
                if op == 'raw' and kt != 0:
                    pl_ap = rawt[:, i, :]
                elif op == 'raw':
                    pl = plane_pool.tile([128, N_PER], bf16, tag="plane",
                                         name="pl", bufs=8)
                    nc.vector.tensor_tensor(pl[:], byt[:, i, :], sct,
                                            mybir.AluOpType.mult)
                    pl_ap = pl[:]
                elif op == 'ge':
                    # split is_ge (4x int mode) + mult (2x): the fused
                    # scalar_tensor_tensor form supports no DVE perf modes
                    # and runs at 1x
                    v = plane_pool.tile([128, N_PER], i16, tag="planei",
                                        name="v", bufs=6)
                    nc.vector.tensor_scalar(v[:], byt[:, i, :], imm, None,
                                            mybir.AluOpType.is_ge)
                    pl = plane_pool.tile([128, N_PER], bf16, tag="plane",
                                         name="pl", bufs=8)
                    nc.vector.tensor_tensor(pl[:], v[:], sct,
                                            mybir.AluOpType.mult)
                    pl_ap = pl[:]
                else:
                    v = plane_pool.tile([128, N_PER], i16, tag="planei",
                                        name="v", bufs=6)
                    nc.vector.tensor_scalar(v[:], byt[:, i, :], imm, None,
                                            _ALU[op])
                    pl = plane_pool.tile([128, N_PER], bf16, tag="plane",
                                         name="pl", bufs=8)
                    nc.vector.tensor_tensor(pl[:], v[:], sct,
                                            mybir.AluOpType.mult)
                    pl_ap = pl[:]
                for ci, (o, w) in enumerate(NCHUNKS):
                    nc.tensor.matmul(
                        psums[ci][:, :w], xd_t[:, bi, :], pl_ap[:, o:o + w],
                        start=(bi == 0), stop=(bi == N_BLOCKS - 1),
                    )
                if bi == 0:
                    # correction matmuls accumulate early (order-free in PSUM)
                    # so they don't serialize at the tail
                    for ci, (o, w) in enumerate(NCHUNKS):
                        nc.tensor.matmul(
                            psums[ci][:, :w], cl_t[:], cr_t[:, o:o + w],
                            start=False, stop=False, skip_group_check=True,
                        )
                bi += 1
        assert bi == N_BLOCKS

        for ci, (o, w) in enumerate(NCHUNKS):
            nc.vector.tensor_copy(out_sb[:, o:o + w], psums[ci][:, :w])
            nc.sync.dma_start(out=out[:, o:o + w], in_=out_sb[:, o:o + w])

    nc.compile()
    return nc


_NC_CACHE = None


def _get_nc():
    global _NC_CACHE
    if _NC_CACHE is None:
        _NC_CACHE = build_nc()
    return _NC_CACHE


def _host_prep(x, W_high_packed, W_low_packed, scales_high, scales_low,
               col_indices, bias):
    """Build per-core input maps (all small tensors precomputed on host)."""
    x = np.asarray(x, np.float32)
    Wh = np.asarray(W_high_packed, np.int32)
    Wl = np.asarray(W_low_packed, np.int32)
    sh = np.asarray(scales_high, np.float32)
    sl = np.asarray(scales_low, np.float32)
    ci = np.asarray(col_indices, np.int64)
    bias = np.asarray(bias, np.float32)
    N = OUT_FEATURES

    x_perm = x[:, ci]
    x_regions = {'high': x_perm[:, :N_HIGH], 'low': x_perm[:, N_HIGH:N_HIGH + N_LOW]}

    # bytes, [128, t, i, N] int16
    bh_full = np.ascontiguousarray(
        Wh.astype(np.int16).reshape(2, 128, 3, N).transpose(1, 0, 2, 3))
    bl_full = np.ascontiguousarray(
        Wl.astype(np.int16).reshape(3, 128, 5, N).transpose(1, 0, 2, 3))

    # scale tiles [128, 5, N]
    p = np.arange(128)
    st_full = np.empty((128, 5, N), np.float32)
    for t in range(2):
        st_full[:, t] = sh[4 * t + p // 32]
    for t in range(3):
        st_full[:, 2 + t] = sl[8 * t + p // 16]
    st_full = st_full.astype(BF16)

    # xdup [128, N_BLOCKS, M] bf16 (replicated across cores)
    xdup = np.zeros((128, N_BLOCKS, M), np.float32)
    bi = 0
    for kt, (region, t) in enumerate(KTILES):
        planes = ktile_planes(kt, region)
        vper = 4 if region == 'high' else 8
        xr = x_regions[region]
        for (_i, _op, _imm, coeffs) in planes:
            for (j, c) in coeffs:
                k = vper * (128 * t + p) + j
                xdup[:, bi, :] += c * xr[:, k].T
            bi += 1
    xdup = xdup.astype(BF16)

    # correction matmul (f32): rows 0..31 = -h_G * group-sums of x, row 32 = bias row
    Xs = x_perm.reshape(M, 32, GROUP).sum(-1)  # [M, 32]
    h = np.array([31.0] * 8 + [15.0] * 24, np.float32)
    corr_lhsT = np.concatenate(
        [-(h[:, None] * Xs.T), np.ones((1, M), np.float32)], 0)  # [33, M]
    all_scales = np.concatenate([sh, sl], 0)  # [32, N]
    corr_rhs_full = np.concatenate([all_scales, bias[None]], 0)  # [33, N]

    in_maps = []
    for c in range(N_CORES):
        nsl = slice(c * N_PER, (c + 1) * N_PER)
        in_maps.append({
            "bytes_high": np.ascontiguousarray(bh_full[..., nsl]),
            "bytes_low": np.ascontiguousarray(bl_full[..., nsl]),
            "scale_tiles": np.ascontiguousarray(st_full[..., nsl]),
            "xdup": xdup,
            "corr_lhsT": corr_lhsT,
            "corr_rhs": np.ascontiguousarray(corr_rhs_full[:, nsl]),
        })
    return in_maps


def kernel(**inputs):
    nc = _get_nc()
    in_maps = _host_prep(**inputs)
    res = run_bass_kernel_spmd(nc, in_maps, core_ids=list(range(N_CORES)))
    return np.concatenate([r["out"] for r in res.results], axis=1)



# revision 17
# speedup vs baseline: 1.1363x; 1.1363x over previous
# Bass/Trainium2 kernel for nn_CPRPackedLinear (mixed 6-bit/5-bit packed
# quantized linear layer), tensor-parallel over out_features on 8 NeuronCores.
#
# Math: out = x_perm[:, :1024] @ deq6(W_high) + x_perm[:, 1024:] @ deq5(W_low) + bias
# where deq = (unpack_bits(packed) - half_range) * group_scale.
#
# Device strategy (per core, N-shard of 1376 columns):
#   The bit-unpack is decomposed into "planes": each packed byte b contributes
#   bitfields ((b >> s) or (b & m)) that enter the matmul as extra contraction
#   rows, with the recombination coefficients folded into a host-prepared
#   duplicated-x stationary operand. Each plane is ONE fused DVE op
#   (scalar_tensor_tensor): plane = (bytes op imm) * scale_tile, which also
#   applies the per-(group, column) dequant scale. The (-half*scale) offsets
#   and the bias enter via one small correction matmul. A single PSUM
#   accumulation chain per 512-column chunk sums everything.
import numpy as np
import ml_dtypes

import concourse.bass as bass
import concourse.mybir as mybir
from concourse import bacc
from concourse.tile import TileContext
from concourse.bass_utils import run_bass_kernel_spmd

BF16 = ml_dtypes.bfloat16

N_CORES = 8
M = 64
OUT_FEATURES = 11008
N_PER = OUT_FEATURES // N_CORES  # 1376
N_HIGH = 1024
N_LOW = 3072
GROUP = 128

# (byte_idx, op, imm, [(value_offset_j, coeff), ...])
# 'raw' planes are the bytes themselves (no bit op needed); masked fields are
# recovered linearly: b>>c = (b - (b & (2^c-1))) / 2^c, folded into coeffs.
# raw planes first (they batch into one op per K-tile and unblock the PE
# earliest), then the masked planes
HIGH_PLANES = [
    (0, 'raw', 0, [(1, 1 / 64)]),
    (1, 'raw', 0, [(2, 1 / 16)]),
    (2, 'raw', 0, [(3, 1 / 4)]),
    (0, 'and', 63, [(0, 1.0), (1, -1 / 64)]),
    (1, 'and', 15, [(1, 4.0), (2, -1 / 16)]),
    (2, 'and', 3, [(2, 16.0), (3, -1 / 4)]),
]
LOW_PLANES = [
    (0, 'raw', 0, [(1, 1 / 32)]),
    (1, 'raw', 0, [(2, 1 / 4)]),
    (2, 'raw', 0, [(4, 1 / 16)]),
    (3, 'raw', 0, [(6, 1 / 64)]),
    (4, 'raw', 0, [(7, 1 / 8)]),
    (0, 'and', 31, [(0, 1.0), (1, -1 / 32)]),
    (1, 'and', 3, [(1, 8.0), (2, -1 / 4)]),
    # bit 7 of b1 via an arith compare: one fused (b>=128)*scale op
    (1, 'ge', 128, [(3, 1.0), (2, -32.0)]),
    (2, 'and', 15, [(3, 2.0), (4, -1 / 16)]),
    (3, 'and', 1, [(4, 16.0), (5, -1 / 2)]),
    (3, 'and', 63, [(5, 1 / 2), (6, -1 / 64)]),
    (4, 'and', 7, [(6, 4.0), (7, -1 / 8)]),
]
# first K-tile: per-byte-row order (follows DMA arrival), no raw batching
HIGH_PLANES_KT0 = [
    (0, 'raw', 0, [(1, 1 / 64)]),
    (0, 'and', 63, [(0, 1.0), (1, -1 / 64)]),
    (1, 'raw', 0, [(2, 1 / 16)]),
    (1, 'and', 15, [(1, 4.0), (2, -1 / 16)]),
    (2, 'raw', 0, [(3, 1 / 4)]),
    (2, 'and', 3, [(2, 16.0), (3, -1 / 4)]),
]

KTILES = [('high', 0), ('high', 1), ('low', 0), ('low', 1), ('low', 2)]
N_BLOCKS = 2 * len(HIGH_PLANES) + 3 * len(LOW_PLANES)  # 48
NCHUNKS = [(1024, 352), (0, 512), (512, 512)]


def ktile_planes(kt, region):
    if kt == 0:
        return HIGH_PLANES_KT0
    return HIGH_PLANES if region == 'high' else LOW_PLANES

_ALU = {
    'and': mybir.AluOpType.bitwise_and,
    'shr': mybir.AluOpType.logical_shift_right,
}


def build_nc():
    nc = bacc.Bacc(None, name="cpr_packed_linear", enable_partition_id=False)
    f32 = mybir.dt.float32
    bf16 = mybir.dt.bfloat16
    i16 = mybir.dt.int16

    bh = nc.dram_tensor("bytes_high", [128, 2, 3, N_PER], i16, kind="ExternalInput")
    bl = nc.dram_tensor("bytes_low", [128, 3, 5, N_PER], i16, kind="ExternalInput")
    st = nc.dram_tensor("scale_tiles", [128, 5, N_PER], bf16, kind="ExternalInput")
    xd = nc.dram_tensor("xdup", [128, N_BLOCKS, M], bf16, kind="ExternalInput")
    cl = nc.dram_tensor("corr_lhsT", [33, M], f32, kind="ExternalInput")
    cr = nc.dram_tensor("corr_rhs", [33, N_PER], f32, kind="ExternalInput")
    out = nc.dram_tensor("out", [M, N_PER], f32, kind="ExternalOutput")

    with TileContext(nc) as tc, \
         tc.tile_pool(name="const", bufs=1) as const_pool, \
         tc.tile_pool(name="bytes", bufs=3) as bytes_pool, \
         tc.tile_pool(name="scales", bufs=2) as scale_pool, \
         tc.tile_pool(name="planes", bufs=2) as plane_pool, \
         tc.tile_pool(name="psum", bufs=1, space="PSUM") as psum_pool:

        # small/critical tensors first, first K-tile bytes split per byte-row
        # so the first plane can start after ~1/3 of the transfer
        byt0 = bytes_pool.tile([128, 3, N_PER], i16, tag="bytes", name="byt0")
        nc.sync.dma_start(out=byt0[:, 0], in_=bh[:, 0, 0])
        sct0 = scale_pool.tile([128, N_PER], bf16, tag="sc", name="sct0")
        nc.scalar.dma_start(out=sct0[:], in_=st[:, 0])
        xd_t = const_pool.tile([128, N_BLOCKS, M], bf16, tag="xd")
        nc.scalar.dma_start(out=xd_t[:, :6], in_=xd[:, :6])
        nc.sync.dma_start(out=byt0[:, 1], in_=bh[:, 0, 1])
        nc.sync.dma_start(out=byt0[:, 2], in_=bh[:, 0, 2])
        nc.scalar.dma_start(out=xd_t[:, 6:], in_=xd[:, 6:])
        cl_t = const_pool.tile([33, M], f32, tag="cl")
        nc.scalar.dma_start(out=cl_t[:], in_=cl[:])
        cr_t = const_pool.tile([33, N_PER], f32, tag="cr")
        nc.scalar.dma_start(out=cr_t[:], in_=cr[:])
        out_sb = const_pool.tile([M, N_PER], f32, tag="outsb")

        psums = [psum_pool.tile([M, w], mybir.dt.float32, tag=f"ps{i}",
                                name=f"ps{i}")
                 for i, (o, w) in enumerate(NCHUNKS)]

        bi = 0
        for kt, (region, t) in enumerate(KTILES):
            nb = 3 if region == 'high' else 5
            if kt == 0:
                byt, sct_full = byt0, sct0
            else:
                byt = bytes_pool.tile([128, nb, N_PER], i16, tag="bytes",
                                      name="byt")
                src = bh[:, t] if region == 'high' else bl[:, t]
                # per-byte-row DMAs: each masked extract can start as soon as
                # its row lands instead of waiting for the whole tile
                for i in range(nb):
                    nc.sync.dma_start(out=byt[:, i], in_=src[:, i])
                sct_full = scale_pool.tile([128, N_PER], bf16, tag="sc",
                                           name="sct")
                nc.scalar.dma_start(out=sct_full[:], in_=st[:, kt])
            sct = sct_full[:]

            planes = ktile_planes(kt, region)

            if kt != 0:
                # all raw planes in ONE batched DVE op: bytes * scale with
                # the scale broadcast (step-0) over the byte-row dim
                rawt = plane_pool.tile([128, nb, N_PER], bf16, tag="rawt",
                                       name="rawt", bufs=3)
                sct1 = sct_full[:].rearrange("p (o n) -> p o n", o=1)
                nc.vector.tensor_tensor(rawt[:], byt[:],
                                        sct1.broadcast_to([128, nb, N_PER]),
                                        mybir.AluOpType.mult)

            for (i, op, imm, _coeffs) in planes:
                if op == 'raw' and kt != 0:
                    pl_ap = rawt[:, i, :]
                elif op == 'raw':
                    pl = plane_pool.tile([128, N_PER], bf16, tag="plane",
                                         name="pl", bufs=8)
                    nc.vector.tensor_tensor(pl[:], byt[:, i, :], sct,
                                            mybir.AluOpType.mult)
                    pl_ap = pl[:]
                elif op == 'ge':
                    pl = plane_pool.tile([128, N_PER], bf16, tag="plane",
                                         name="pl", bufs=8)
                    nc.vector.scalar_tensor_tensor(
                        pl[:], byt[:, i, :], float(imm), sct,
                        mybir.AluOpType.is_ge, mybir.AluOpType.mult)
                    pl_ap = pl[:]
                else:
                    v = plane_pool.tile([128, N_PER], i16, tag="planei",
                                        name="v", bufs=6)
                    nc.vector.tensor_scalar(v[:], byt[:, i, :], imm, None,
                                            _ALU[op])
                    pl = plane_pool.tile([128, N_PER], bf16, tag="plane",
                                         name="pl", bufs=8)
                    nc.vector.tensor_tensor(pl[:], v[:], sct,
                                            mybir.AluOpType.mult)
                    pl_ap = pl[:]
                for ci, (o, w) in enumerate(NCHUNKS):
                    nc.tensor.matmul(
                        psums[ci][:, :w], xd_t[:, bi, :], pl_ap[:, o:o + w],
                        start=(bi == 0), stop=(bi == N_BLOCKS - 1),
                    )
                if bi == 0:
                    # correction matmuls accumulate early (order-free in PSUM)
                    # so they don't serialize at the tail
                    for ci, (o, w) in enumerate(NCHUNKS):
                        nc.tensor.matmul(
                            psums[ci][:, :w], cl_t[:], cr_t[:, o:o + w],
                            start=False, stop=False, skip_group_check=True,
                        )
                bi += 1
        assert bi == N_BLOCKS

        for ci, (o, w) in enumerate(NCHUNKS):
            nc.vector.tensor_copy(out_sb[:, o:o + w], psums[ci][:, :w])
            nc.sync.dma_start(out=out[:, o:o + w], in_=out_sb[:, o:o + w])

    nc.compile()
    return nc


_NC_CACHE = None


def _get_nc():
    global _NC_CACHE
    if _NC_CACHE is None:
        _NC_CACHE = build_nc()
    return _NC_CACHE


def _host_prep(x, W_high_packed, W_low_packed, scales_high, scales_low,
               col_indices, bias):
    """Build per-core input maps (all small tensors precomputed on host)."""
    x = np.asarray(x, np.float32)
    Wh = np.asarray(W_high_packed, np.int32)
    Wl = np.asarray(W_low_packed, np.int32)
    sh = np.asarray(scales_high, np.float32)
    sl = np.asarray(scales_low, np.float32)
    ci = np.asarray(col_indices, np.int64)
    bias = np.asarray(bias, np.float32)
    N = OUT_FEATURES

    x_perm = x[:, ci]
    x_regions = {'high': x_perm[:, :N_HIGH], 'low': x_perm[:, N_HIGH:N_HIGH + N_LOW]}

    # bytes, [128, t, i, N] int16
    bh_full = np.ascontiguousarray(
        Wh.astype(np.int16).reshape(2, 128, 3, N).transpose(1, 0, 2, 3))
    bl_full = np.ascontiguousarray(
        Wl.astype(np.int16).reshape(3, 128, 5, N).transpose(1, 0, 2, 3))

    # scale tiles [128, 5, N]
    p = np.arange(128)
    st_full = np.empty((128, 5, N), np.float32)
    for t in range(2):
        st_full[:, t] = sh[4 * t + p // 32]
    for t in range(3):
        st_full[:, 2 + t] = sl[8 * t + p // 16]
    st_full = st_full.astype(BF16)

    # xdup [128, N_BLOCKS, M] bf16 (replicated across cores)
    xdup = np.zeros((128, N_BLOCKS, M), np.float32)
    bi = 0
    for kt, (region, t) in enumerate(KTILES):
        planes = ktile_planes(kt, region)
        vper = 4 if region == 'high' else 8
        xr = x_regions[region]
        for (_i, _op, _imm, coeffs) in planes:
            for (j, c) in coeffs:
                k = vper * (128 * t + p) + j
                xdup[:, bi, :] += c * xr[:, k].T
            bi += 1
    xdup = xdup.astype(BF16)

    # correction matmul (f32): rows 0..31 = -h_G * group-sums of x, row 32 = bias row
    Xs = x_perm.reshape(M, 32, GROUP).sum(-1)  # [M, 32]
    h = np.array([31.0] * 8 + [15.0] * 24, np.float32)
    corr_lhsT = np.concatenate(
        [-(h[:, None] * Xs.T), np.ones((1, M), np.float32)], 0)  # [33, M]
    all_scales = np.concatenate([sh, sl], 0)  # [32, N]
    corr_rhs_full = np.concatenate([all_scales, bias[None]], 0)  # [33, N]

    in_maps = []
    for c in range(N_CORES):
        nsl = slice(c * N_PER, (c + 1) * N_PER)
        in_maps.append({
            "bytes_high": np.ascontiguousarray(bh_full[..., nsl]),
            "bytes_low": np.ascontiguousarray(bl_full[..., nsl]),
            "scale_tiles": np.ascontiguousarray(st_full[..., nsl]),
            "xdup": xdup,
            "corr_lhsT": corr_lhsT,
            "corr_rhs": np.ascontiguousarray(corr_rhs_full[:, nsl]),
        })
    return in_maps


def kernel(**inputs):
    nc = _get_nc()
    in_maps = _host_prep(**inputs)
    res = run_bass_kernel_spmd(nc, in_maps, core_ids=list(range(N_CORES)))
    return np.concatenate([r["out"] for r in res.results], axis=1)



# revision 18
# speedup vs baseline: 1.1516x; 1.0135x over previous
# Bass/Trainium2 kernel for nn_CPRPackedLinear (mixed 6-bit/5-bit packed
# quantized linear layer), tensor-parallel over out_features on 8 NeuronCores.
#
# Math: out = x_perm[:, :1024] @ deq6(W_high) + x_perm[:, 1024:] @ deq5(W_low) + bias
# where deq = (unpack_bits(packed) - half_range) * group_scale.
#
# Device strategy (per core, N-shard of 1376 columns):
#   The bit-unpack is decomposed into "planes": each packed byte b contributes
#   bitfields ((b >> s) or (b & m)) that enter the matmul as extra contraction
#   rows, with the recombination coefficients folded into a host-prepared
#   duplicated-x stationary operand. Each plane is ONE fused DVE op
#   (scalar_tensor_tensor): plane = (bytes op imm) * scale_tile, which also
#   applies the per-(group, column) dequant scale. The (-half*scale) offsets
#   and the bias enter via one small correction matmul. A single PSUM
#   accumulation chain per 512-column chunk sums everything.
import numpy as np
import ml_dtypes

import concourse.bass as bass
import concourse.mybir as mybir
from concourse import bacc
from concourse.tile import TileContext
from concourse.bass_utils import run_bass_kernel_spmd

BF16 = ml_dtypes.bfloat16

N_CORES = 8
M = 64
OUT_FEATURES = 11008
N_PER = OUT_FEATURES // N_CORES  # 1376
N_HIGH = 1024
N_LOW = 3072
GROUP = 128

# (byte_idx, op, imm, [(value_offset_j, coeff), ...])
# 'raw' planes are the bytes themselves (no bit op needed); masked fields are
# recovered linearly: b>>c = (b - (b & (2^c-1))) / 2^c, folded into coeffs.
# raw planes first (they batch into one op per K-tile and unblock the PE
# earliest), then the masked planes
HIGH_PLANES = [
    (0, 'raw', 0, [(1, 1 / 64)]),
    (1, 'raw', 0, [(2, 1 / 16)]),
    (2, 'raw', 0, [(3, 1 / 4)]),
    (0, 'and', 63, [(0, 1.0), (1, -1 / 64)]),
    (1, 'and', 15, [(1, 4.0), (2, -1 / 16)]),
    (2, 'and', 3, [(2, 16.0), (3, -1 / 4)]),
]
LOW_PLANES = [
    (0, 'raw', 0, [(1, 1 / 32)]),
    (1, 'raw', 0, [(2, 1 / 4)]),
    (2, 'raw', 0, [(4, 1 / 16)]),
    (3, 'raw', 0, [(6, 1 / 64)]),
    (4, 'raw', 0, [(7, 1 / 8)]),
    (0, 'and', 31, [(0, 1.0), (1, -1 / 32)]),
    (1, 'and', 3, [(1, 8.0), (2, -1 / 4)]),
    # bit 7 of b1 via an arith compare: one fused (b>=128)*scale op
    (1, 'ge', 128, [(3, 1.0), (2, -32.0)]),
    (2, 'and', 15, [(3, 2.0), (4, -1 / 16)]),
    (3, 'and', 1, [(4, 16.0), (5, -1 / 2)]),
    (3, 'and', 63, [(5, 1 / 2), (6, -1 / 64)]),
    (4, 'and', 7, [(6, 4.0), (7, -1 / 8)]),
]
# first K-tile: per-byte-row order (follows DMA arrival), no raw batching
HIGH_PLANES_KT0 = [
    (0, 'raw', 0, [(1, 1 / 64)]),
    (0, 'and', 63, [(0, 1.0), (1, -1 / 64)]),
    (1, 'raw', 0, [(2, 1 / 16)]),
    (1, 'and', 15, [(1, 4.0), (2, -1 / 16)]),
    (2, 'raw', 0, [(3, 1 / 4)]),
    (2, 'and', 3, [(2, 16.0), (3, -1 / 4)]),
]

KTILES = [('high', 0), ('high', 1), ('low', 0), ('low', 1), ('low', 2)]
N_BLOCKS = 2 * len(HIGH_PLANES) + 3 * len(LOW_PLANES)  # 48
NCHUNKS = [(1024, 352), (0, 512), (512, 512)]


def ktile_planes(kt, region):
    if kt == 0:
        return HIGH_PLANES_KT0
    return HIGH_PLANES if region == 'high' else LOW_PLANES

_ALU = {
    'and': mybir.AluOpType.bitwise_and,
    'shr': mybir.AluOpType.logical_shift_right,
}


def build_nc():
    nc = bacc.Bacc(None, name="cpr_packed_linear", enable_partition_id=False)
    f32 = mybir.dt.float32
    bf16 = mybir.dt.bfloat16
    i16 = mybir.dt.int16

    bh = nc.dram_tensor("bytes_high", [128, 2, 3, N_PER], i16, kind="ExternalInput")
    bl = nc.dram_tensor("bytes_low", [128, 3, 5, N_PER], i16, kind="ExternalInput")
    st = nc.dram_tensor("scale_tiles", [128, 5, N_PER], bf16, kind="ExternalInput")
    xd = nc.dram_tensor("xdup", [128, N_BLOCKS, M], bf16, kind="ExternalInput")
    cl = nc.dram_tensor("corr_lhsT", [33, M], f32, kind="ExternalInput")
    cr = nc.dram_tensor("corr_rhs", [33, N_PER], f32, kind="ExternalInput")
    out = nc.dram_tensor("out", [M, N_PER], f32, kind="ExternalOutput")

    with TileContext(nc) as tc, \
         tc.tile_pool(name="const", bufs=1) as const_pool, \
         tc.tile_pool(name="bytes", bufs=4) as bytes_pool, \
         tc.tile_pool(name="scales", bufs=2) as scale_pool, \
         tc.tile_pool(name="planes", bufs=2) as plane_pool, \
         tc.tile_pool(name="psum", bufs=1, space="PSUM") as psum_pool:

        # small/critical tensors first, first K-tile bytes split per byte-row
        # so the first plane can start after ~1/3 of the transfer
        byt0 = bytes_pool.tile([128, 3, N_PER], i16, tag="bytes", name="byt0")
        nc.sync.dma_start(out=byt0[:, 0], in_=bh[:, 0, 0])
        sct0 = scale_pool.tile([128, N_PER], bf16, tag="sc", name="sct0")
        nc.scalar.dma_start(out=sct0[:], in_=st[:, 0])
        xd_t = const_pool.tile([128, N_BLOCKS, M], bf16, tag="xd")
        nc.scalar.dma_start(out=xd_t[:, :6], in_=xd[:, :6])
        nc.sync.dma_start(out=byt0[:, 1], in_=bh[:, 0, 1])
        nc.sync.dma_start(out=byt0[:, 2], in_=bh[:, 0, 2])
        nc.scalar.dma_start(out=xd_t[:, 6:], in_=xd[:, 6:])
        cl_t = const_pool.tile([33, M], f32, tag="cl")
        nc.scalar.dma_start(out=cl_t[:], in_=cl[:])
        cr_t = const_pool.tile([33, N_PER], f32, tag="cr")
        nc.scalar.dma_start(out=cr_t[:], in_=cr[:])
        out_sb = const_pool.tile([M, N_PER], f32, tag="outsb")

        psums = [psum_pool.tile([M, w], mybir.dt.float32, tag=f"ps{i}",
                                name=f"ps{i}")
                 for i, (o, w) in enumerate(NCHUNKS)]

        bi = 0
        for kt, (region, t) in enumerate(KTILES):
            nb = 3 if region == 'high' else 5
            if kt == 0:
                byt, sct_full = byt0, sct0
            else:
                byt = bytes_pool.tile([128, nb, N_PER], i16, tag="bytes",
                                      name="byt")
                src = bh[:, t] if region == 'high' else bl[:, t]
                # per-byte-row DMAs: each masked extract can start as soon as
                # its row lands instead of waiting for the whole tile
                for i in range(nb):
                    nc.sync.dma_start(out=byt[:, i], in_=src[:, i])
                sct_full = scale_pool.tile([128, N_PER], bf16, tag="sc",
                                           name="sct")
                nc.scalar.dma_start(out=sct_full[:], in_=st[:, kt])
            sct = sct_full[:]

            planes = ktile_planes(kt, region)

            if kt != 0:
                # all raw planes in ONE batched DVE op: bytes * scale with
                # the scale broadcast (step-0) over the byte-row dim
                rawt = plane_pool.tile([128, nb, N_PER], bf16, tag="rawt",
                                       name="rawt", bufs=4)
                sct1 = sct_full[:].rearrange("p (o n) -> p o n", o=1)
                nc.vector.tensor_tensor(rawt[:], byt[:],
                                        sct1.broadcast_to([128, nb, N_PER]),
                                        mybir.AluOpType.mult)

            for (i, op, imm, _coeffs) in planes:
                if op == 'raw' and kt != 0:
                    pl_ap = rawt[:, i, :]
                elif op == 'raw':
                    pl = plane_pool.tile([128, N_PER], bf16, tag="plane",
                                         name="pl", bufs=12)
                    nc.vector.tensor_tensor(pl[:], byt[:, i, :], sct,
                                            mybir.AluOpType.mult)
                    pl_ap = pl[:]
                elif op == 'ge':
                    pl = plane_pool.tile([128, N_PER], bf16, tag="plane",
                                         name="pl", bufs=12)
                    nc.vector.scalar_tensor_tensor(
                        pl[:], byt[:, i, :], float(imm), sct,
                        mybir.AluOpType.is_ge, mybir.AluOpType.mult)
                    pl_ap = pl[:]
                else:
                    v = plane_pool.tile([128, N_PER], i16, tag="planei",
                                        name="v", bufs=10)
                    nc.vector.tensor_scalar(v[:], byt[:, i, :], imm, None,
                                            _ALU[op])
                    pl = plane_pool.tile([128, N_PER], bf16, tag="plane",
                                         name="pl", bufs=12)
                    nc.vector.tensor_tensor(pl[:], v[:], sct,
                                            mybir.AluOpType.mult)
                    pl_ap = pl[:]
                for ci, (o, w) in enumerate(NCHUNKS):
                    nc.tensor.matmul(
                        psums[ci][:, :w], xd_t[:, bi, :], pl_ap[:, o:o + w],
                        start=(bi == 0), stop=(bi == N_BLOCKS - 1),
                    )
                if bi == 0:
                    # correction matmuls accumulate early (order-free in PSUM)
                    # so they don't serialize at the tail
                    for ci, (o, w) in enumerate(NCHUNKS):
                        nc.tensor.matmul(
                            psums[ci][:, :w], cl_t[:], cr_t[:, o:o + w],
                            start=False, stop=False, skip_group_check=True,
                        )
                bi += 1
        assert bi == N_BLOCKS

        for ci, (o, w) in enumerate(NCHUNKS):
            nc.vector.tensor_copy(out_sb[:, o:o + w], psums[ci][:, :w])
            nc.sync.dma_start(out=out[:, o:o + w], in_=out_sb[:, o:o + w])

    nc.compile()
    return nc


_NC_CACHE = None


def _get_nc():
    global _NC_CACHE
    if _NC_CACHE is None:
        _NC_CACHE = build_nc()
    return _NC_CACHE


def _host_prep(x, W_high_packed, W_low_packed, scales_high, scales_low,
               col_indices, bias):
    """Build per-core input maps (all small tensors precomputed on host)."""
    x = np.asarray(x, np.float32)
    Wh = np.asarray(W_high_packed, np.int32)
    Wl = np.asarray(W_low_packed, np.int32)
    sh = np.asarray(scales_high, np.float32)
    sl = np.asarray(scales_low, np.float32)
    ci = np.asarray(col_indices, np.int64)
    bias = np.asarray(bias, np.float32)
    N = OUT_FEATURES

    x_perm = x[:, ci]
    x_regions = {'high': x_perm[:, :N_HIGH], 'low': x_perm[:, N_HIGH:N_HIGH + N_LOW]}

    # bytes, [128, t, i, N] int16
    bh_full = np.ascontiguousarray(
        Wh.astype(np.int16).reshape(2, 128, 3, N).transpose(1, 0, 2, 3))
    bl_full = np.ascontiguousarray(
        Wl.astype(np.int16).reshape(3, 128, 5, N).transpose(1, 0, 2, 3))

    # scale tiles [128, 5, N]
    p = np.arange(128)
    st_full = np.empty((128, 5, N), np.float32)
    for t in range(2):
        st_full[:, t] = sh[4 * t + p // 32]
    for t in range(3):
        st_full[:, 2 + t] = sl[8 * t + p // 16]
    st_full = st_full.astype(BF16)

    # xdup [128, N_BLOCKS, M] bf16 (replicated across cores)
    xdup = np.zeros((128, N_BLOCKS, M), np.float32)
    bi = 0
    for kt, (region, t) in enumerate(KTILES):
        planes = ktile_planes(kt, region)
        vper = 4 if region == 'high' else 8
        xr = x_regions[region]
        for (_i, _op, _imm, coeffs) in planes:
            for (j, c) in coeffs:
                k = vper * (128 * t + p) + j
                xdup[:, bi, :] += c * xr[:, k].T
            bi += 1
    xdup = xdup.astype(BF16)

    # correction matmul (f32): rows 0..31 = -h_G * group-sums of x, row 32 = bias row
    Xs = x_perm.reshape(M, 32, GROUP).sum(-1)  # [M, 32]
    h = np.array([31.0] * 8 + [15.0] * 24, np.float32)
    corr_lhsT = np.concatenate(
        [-(h[:, None] * Xs.T), np.ones((1, M), np.float32)], 0)  # [33, M]
    all_scales = np.concatenate([sh, sl], 0)  # [32, N]
    corr_rhs_full = np.concatenate([all_scales, bias[None]], 0)  # [33, N]

    in_maps = []
    for c in range(N_CORES):
        nsl = slice(c * N_PER, (c + 1) * N_PER)
        in_maps.append({
            "bytes_high": np.ascontiguousarray(bh_full[..., nsl]),
            "bytes_low": np.ascontiguousarray(bl_full[..., nsl]),
            "scale_tiles": np.ascontiguousarray(st_full[..., nsl]),
            "xdup": xdup,
            "corr_lhsT": corr_lhsT,
            "corr_rhs": np.ascontiguousarray(corr_rhs_full[:, nsl]),
        })
    return in_maps


def kernel(**inputs):
    nc = _get_nc()
    in_maps = _host_prep(**inputs)
    res = run_bass_kernel_spmd(nc, in_maps, core_ids=list(range(N_CORES)))
    return np.concatenate([r["out"] for r in res.results], axis=1)

